# revision 22
# baseline (speedup 1.0000x reference)
"""COIL scoring kernel for Trainium2, sharded over 8 NeuronCores.

Sharding: data-parallel over documents (Bd=256 -> 32 docs/core). Each core:
  - projects its doc tokens with W_tok (+ReLU) and token-0 with W_cls
  - projects the (replicated) query tokens the same way
  - computes token-level scores via [97 x 128 x 512] matmuls where the
    exact-match id constraint is encoded as extra one-hot "digit" dimensions
    (base-32 digits of the id, scaled by 64) plus a -8192 penalty constant:
    equal ids add 0 to the score, any mismatch adds <= -4096, so
    relu(max_j score) == max_j (score * exact_match) for this data regime.
  - doc projection and token-score matmuls interleave per doc-chunk to keep
    the PE dense (HAM at full clock; dummy warmup matmuls pre-promote the
    clock while DMAs stream); DVE does paired [128,8,128] max reduces
    straight from PSUM (reduce rate is 1x regardless of staging, so the
    ~39us DVE scan is the steady-state bound and everything overlaps it).
  - each doc-pair's finished maxes fold into the final scores immediately
    (selector matmul accumulating onto the CLS matmuls in one PSUM tile).
Host: pre-packs bf16 operands in SBUF layout (one <=128-descriptor DMA per
tensor), gathers per-core [32,32] score tiles, computes softmax loss.
"""

import os
import numpy as np
import ml_dtypes

import concourse.bass as bass
import concourse.bacc as bacc
import concourse.mybir as mybir
from concourse import tile
from concourse.bass_utils import run_bass_kernel_spmd

BF16 = mybir.dt.bfloat16
FP8 = mybir.dt.float8e4
F32 = mybir.dt.float32
RELU = mybir.ActivationFunctionType.Relu
IDENT = mybir.ActivationFunctionType.Identity
COPY = mybir.ActivationFunctionType.Copy
AX_X = mybir.AxisListType.X
PSUM = bass.MemorySpace.PSUM

N_CORES = 8
Bq, Lq, Bd, Ld, H, Dt, Dc = 32, 32, 256, 128, 768, 32, 768
DPC = Bd // N_CORES          # docs per core = 32
NDT = DPC * Ld               # doc tokens per core = 4096
NQT = Bq * Lq                # query tokens = 1024
KC = H // 128                # contraction chunks = 6
EXT = 65                     # id-encoding extension rows (32+32 digits + const)
KTOT = Dt + EXT              # 97
CONE = 64.0                  # one-hot scale; CONE^2 = 4096 penalty unit
PEN = -2.0 * CONE * CONE     # -8192
NT = NQT // 128              # query-token tiles = 8
NN = NDT // 512              # doc-token 512-chunks = 8
NP = NN // 2                 # paired 1024-chunks = 4
TRAIN_GROUP_SIZE = 8

bf16 = ml_dtypes.bfloat16
fp8 = ml_dtypes.float8_e4m3

_prog = None
last_results = None          # BassKernelResults of the most recent run


def _build():
    nc = bacc.Bacc("TRN2", target_bir_lowering=False, debug=False)

    dhT_d = nc.dram_tensor("dhT", [128, NN, KC, 512], BF16, kind="ExternalInput")
    qhT_d = nc.dram_tensor("qhT", [128, KC, NQT], BF16, kind="ExternalInput")
    h0T_d = nc.dram_tensor("h0T", [128, KC, 2 * Bq], BF16, kind="ExternalInput")
    wtok_d = nc.dram_tensor("wtok", [128, KC, Dt], BF16, kind="ExternalInput")
    wcls_d = nc.dram_tensor("wcls", [128, KC, Dc], BF16, kind="ExternalInput")
    btok_d = nc.dram_tensor("btok", [Dt, 1], F32, kind="ExternalInput")
    bcls_d = nc.dram_tensor("bcls", [128, KC], F32, kind="ExternalInput")
    extq_d = nc.dram_tensor("extq", [EXT, NQT], BF16, kind="ExternalInput")
    extd_d = nc.dram_tensor("extd", [EXT, NDT], BF16, kind="ExternalInput")
    wsel_d = nc.dram_tensor("wsel", [128, NT, Bq], BF16, kind="ExternalInput")
    ident_d = nc.inline_tensor(np.eye(2 * Bq, dtype=bf16), name="ident64")
    out_d = nc.dram_tensor("scores_out", [Bq, DPC], F32, kind="ExternalOutput")

    with tile.TileContext(nc) as tc:
        with (
            tc.tile_pool(name="big", bufs=1) as bigp,
            tc.tile_pool(name="psP", bufs=2, space=PSUM) as psPp,
            tc.tile_pool(name="psS", bufs=2, space=PSUM) as psSp,
            tc.tile_pool(name="psC", bufs=1, space=PSUM) as psCp,
            tc.tile_pool(name="psF", bufs=1, space=PSUM) as psFp,
        ):
            # ---- loads: host-packed, one DMA per tensor, priority order.
            # sync ring: small consts -> wcls (cls unblocks PE early) -> qhT
            wtok_sb = bigp.tile([128, KC, Dt], BF16, tag="wtok")
            wcls_sb = bigp.tile([128, KC, Dc], BF16, tag="wcls")
            h0T_sb = bigp.tile([128, KC, 2 * Bq], BF16, tag="h0T")
            btok_sb = bigp.tile([Dt, 1], F32, tag="btok")
            bcls_sb = bigp.tile([128, KC], F32, tag="bcls")
            wsel_sb = bigp.tile([128, NT, Bq], BF16, tag="wsel")
            ident_sb = bigp.tile([2 * Bq, 2 * Bq], BF16, tag="ident")
            QT = bigp.tile([128, NQT], BF16, tag="QT")
            qhT_sb = bigp.tile([128, KC, NQT], BF16, tag="qhT")
            DT = bigp.tile([128, NDT], BF16, tag="DT")
            dhq = bigp.tile([128, NN, KC, 512], BF16, tag="dh")
            nc.sync.dma_start(out=wtok_sb[:, :, :], in_=wtok_d[:, :, :])
            nc.sync.dma_start(out=h0T_sb[:, :, :], in_=h0T_d[:, :, :])
            nc.sync.dma_start(out=btok_sb[:, :], in_=btok_d[:, :])
            nc.sync.dma_start(out=bcls_sb[:, :], in_=bcls_d[:, :])
            nc.sync.dma_start(out=ident_sb[:, :], in_=ident_d[:, :])
            nc.sync.dma_start(out=qhT_sb[:, :, :], in_=qhT_d[:, :, :])
            nc.sync.dma_start(out=QT[Dt : Dt + EXT, :], in_=extq_d[:, :])
            nc.sync.dma_start(out=dhq[:, 0, :, :], in_=dhT_d[:, 0, :, :])
            nc.sync.dma_start(out=DT[Dt : Dt + EXT, :], in_=extd_d[:, :])
            nc.sync.dma_start(out=dhq[:, 1, :, :], in_=dhT_d[:, 1, :, :])
            nc.sync.dma_start(out=dhq[:, 2:4, :, :], in_=dhT_d[:, 2:4, :, :])
            nc.sync.dma_start(out=dhq[:, 4:6, :, :], in_=dhT_d[:, 4:6, :, :])
            nc.sync.dma_start(out=wcls_sb[:, :, :], in_=wcls_d[:, :, :])
            nc.sync.dma_start(out=dhq[:, 6:8, :, :], in_=dhT_d[:, 6:8, :, :])
            nc.sync.dma_start(out=wsel_sb[:, :, :], in_=wsel_d[:, :, :])

            # ---- PE warmup: promote HAM to full clock while DMAs stream
            for wi in range(18):
                psw = psPp.tile([Dt, 512], F32, tag="proj")
                nc.tensor.matmul(
                    psw[:, 0:384],
                    lhsT=wtok_sb[:, 0, :],
                    rhs=h0T_sb[:, :, :].rearrange("x c n -> x (c n)"),
                    start=True,
                    stop=True,
                )

            # ---- query projection (first half; second half lands mid-stream)
            def qproj(h):
                ps = psPp.tile([Dt, 512], F32, tag="proj")
                hs = slice(h * 512, (h + 1) * 512)
                for c in range(KC):
                    nc.tensor.matmul(
                        ps[:, :],
                        lhsT=wtok_sb[:, c, :],
                        rhs=qhT_sb[:, c, hs],
                        start=(c == 0),
                        stop=(c == KC - 1),
                    )
                nc.scalar.activation(QT[0:Dt, hs], ps[:, :], RELU, bias=btok_sb[:, 0:1])

            qproj(0)

            # ---- interleaved doc projection + token scores.
            # Groups: two 512-col singles (so DVE starts as soon as the first
            # eighth of doc hidden lands), then three 1024-col pairs.
            tokraw = bigp.tile([128, NT, DPC], BF16, tag="tokraw")
            tok = bigp.tile([128, NT, DPC], BF16, tag="tok")
            psF = psFp.tile([Bq, DPC], F32, tag="fin")
            qd2 = bigp.tile([2 * Bq, Dc], BF16, tag="qd2")
            qdcls = bigp.tile([128, KC, 2 * Bq], BF16, tag="qdcls")
            groups = [(0,), (1,), (2, 3), (4, 5), (6, 7)]

            def dproj(grp):
                for n in grp:
                    ps = psPp.tile([Dt, 512], F32, tag="proj")
                    for c in range(KC):
                        nc.tensor.matmul(
                            ps[:, :],
                            lhsT=wtok_sb[:, c, :],
                            rhs=dhq[:, n, c, :],
                            start=(c == 0),
                            stop=(c == KC - 1),
                        )
                    nc.scalar.activation(
                        DT[0:Dt, n * 512 : (n + 1) * 512],
                        ps[:, :],
                        RELU,
                        bias=btok_sb[:, 0:1],
                    )

            dproj(groups[0])
            for gi, grp in enumerate(groups):
                for t in range(NT):
                    if gi == 0 and t == 4:
                        qproj(1)
                    ts = slice(t * 128, (t + 1) * 128)
                    psb = psSp.tile([128, 512 * len(grp)], F32, tag="S")
                    for j, n in enumerate(grp):
                        nc.tensor.matmul(
                            psb[:, j * 512 : (j + 1) * 512],
                            lhsT=QT[0:KTOT, ts],
                            rhs=DT[0:KTOT, n * 512 : (n + 1) * 512],
                            start=True,
                            stop=True,
                        )
                    nc.vector.reduce_max(
                        tokraw[:, t, grp[0] * 4 : (grp[-1] + 1) * 4],
                        psb[:, :].rearrange("x (a b) -> x a b", b=Ld),
                        axis=AX_X,
                    )
                if gi + 1 < len(groups):
                    dproj(groups[gi + 1])
                if gi == 0:
                    continue
                # fold the completed 8-doc block into the final scores
                b = gi - 1
                for t in range(NT):
                    psl = slice(b * 8, (b + 1) * 8)
                    nc.scalar.activation(tok[:, t, psl], tokraw[:, t, psl], RELU)
                    nc.tensor.matmul(
                        psF[:, psl],
                        lhsT=wsel_sb[:, t, :],
                        rhs=tok[:, t, psl],
                        start=(b == 0 and t == 0),
                        stop=(b == 3 and t == NT - 1),
                    )
                    if b == 3 and t == 0:
                        # cls scores accumulate here, off the critical tail
                        for m in range(KC):
                            nc.tensor.matmul(
                                psF[:, :],
                                lhsT=qdcls[:, m, 0:Bq],
                                rhs=qdcls[:, m, Bq : 2 * Bq],
                                start=False,
                                stop=False,
                            )
                if gi == 2:
                    # cls projection filler (wcls arrives during the S stream)
                    for half in range(2):
                        psc = psCp.tile([2 * Bq, Dc // 2], F32, tag="mix")
                        cs = slice(half * 384, (half + 1) * 384)
                        for c in range(KC):
                            nc.tensor.matmul(
                                psc[:, :],
                                lhsT=h0T_sb[:, c, :],
                                rhs=wcls_sb[:, c, cs],
                                start=(c == 0),
                                stop=(c == KC - 1),
                            )
                        nc.scalar.activation(qd2[:, cs], psc[:, :], COPY)
                elif gi == 3:
                    # cls transpose filler
                    for m in range(KC):
                        pst = psPp.tile([128, 2 * Bq], BF16, tag="proj")
                        nc.tensor.transpose(
                            pst[:, :], qd2[:, m * 128 : (m + 1) * 128], ident_sb[:, :]
                        )
                        nc.scalar.activation(
                            qdcls[:, m, :], pst[:, :], IDENT, bias=bcls_sb[:, m : m + 1]
                        )

            scr = bigp.tile([Bq, DPC], F32, tag="scr")
            nc.vector.tensor_copy(scr[:, :], psF[:, :])
            nc.sync.dma_start(out=out_d[:, :], in_=scr[:, :])

    nc.compile()
    return nc


def _get_prog():
    global _prog
    if _prog is None:
        _prog = _build()
    return _prog


def _pack(a, p=128):
    """[C*p, N...] -> [p, C, N...] contiguous (SBUF partition-major layout)."""
    c = a.shape[0] // p
    return np.ascontiguousarray(a.reshape((c, p) + a.shape[1:]).swapaxes(0, 1))


def _prep_inputs(qry_hidden, doc_hidden, W_tok, b_tok, W_cls, b_cls,
                 qry_input_ids, doc_input_ids, qry_attention_mask):
    qh = np.asarray(qry_hidden, np.float32)
    dh = np.asarray(doc_hidden, np.float32)
    qids = np.asarray(qry_input_ids, np.int32).reshape(-1)
    dids = np.asarray(doc_input_ids, np.int32)
    amask = np.asarray(qry_attention_mask, np.int32)

    qhT = np.ascontiguousarray(qh.reshape(NQT, H).astype(bf16).T)     # [768, 1024]
    wtok = _pack(np.asarray(W_tok, np.float32).astype(bf16))          # [128,6,32]
    wcls = _pack(np.asarray(W_cls, np.float32).astype(bf16))          # [128,6,768]
    btok = np.asarray(b_tok, np.float32).reshape(Dt, 1)
    bcls = _pack(np.asarray(b_cls, np.float32).reshape(Dc, 1))[:, :, 0]  # [128,6]

    g = np.arange(NQT)
    extq = np.zeros((EXT, NQT), np.float32)
    extq[qids % 32, g] = CONE
    extq[32 + qids // 32, g] = CONE
    extq[64, :] = 1.0
    extq = extq.astype(bf16)

    # query-token weights: qmask with sep position zeroed, i=0 dropped
    sep = amask.sum(1) - 1
    qm = amask.astype(np.float32).copy()
    qm[np.arange(Bq), sep] = 0.0
    w = qm.copy()
    w[:, 0] = 0.0
    wsel = np.zeros((NQT, Bq), np.float32)
    wsel[g, g // Lq] = w.reshape(-1)
    wsel = _pack(wsel.astype(bf16))                                   # [128,8,32]

    qh0 = qh[:, 0, :]
    in_maps = []
    for k in range(N_CORES):
        dsl = slice(k * DPC, (k + 1) * DPC)
        dh_k = dh[dsl].reshape(NDT, H)
        dhT_k = dh_k.astype(bf16).T                                   # [768, 4096]
        # pack to [128, 8(eighth), 6(c), 512]
        dhTP = np.ascontiguousarray(
            dhT_k.reshape(KC, 128, NN, 512).transpose(1, 2, 0, 3)
        )
        h0T_k = _pack(
            np.ascontiguousarray(
                np.concatenate([qh0, dh[dsl, 0, :]], axis=0).astype(bf16).T
            )
        )                                                             # [128,6,64]
        dids_k = dids[dsl].reshape(-1)
        gd = np.arange(NDT)
        extd = np.zeros((EXT, NDT), np.float32)
        extd[dids_k % 32, gd] = CONE
        extd[32 + dids_k // 32, gd] = CONE
        extd[64, :] = PEN
        in_maps.append({
            "dhT": dhTP,
            "qhT": _pack(qhT),
            "h0T": h0T_k,
            "wtok": wtok,
            "wcls": wcls,
            "btok": btok,
            "bcls": bcls,
            "extq": extq,
            "extd": extd.astype(bf16),
            "wsel": wsel,
        })
    return in_maps


def kernel(**inputs):
    global last_results
    nc = _get_prog()
    in_maps = _prep_inputs(**inputs)
    trace = bool(os.environ.get("COIL_TRACE"))
    last_results = run_bass_kernel_spmd(
        nc, in_maps, list(range(N_CORES)), trace=trace
    )
    scores = np.concatenate(
        [last_results.results[k]["scores_out"] for k in range(N_CORES)], axis=1
    ).astype(np.float64)

    labels = np.arange(Bq) * TRAIN_GROUP_SIZE
    m = scores.max(axis=1, keepdims=True)
    lse = m[:, 0] + np.log(np.exp(scores - m).sum(axis=1))
    loss = -(scores[np.arange(Bq), labels] - lse).mean()
    return (
        np.asarray(loss, np.float32),
        scores.reshape(-1).astype(np.float32),
    )


# revision 23
# speedup vs baseline: 1.0283x; 1.0283x over previous
"""COIL scoring kernel for Trainium2, sharded over 8 NeuronCores.

Sharding: data-parallel over documents (Bd=256 -> 32 docs/core). Each core:
  - projects its doc tokens with W_tok (+ReLU) and token-0 with W_cls
  - projects the (replicated) query tokens the same way
  - computes token-level scores via [97 x 128 x 512] matmuls where the
    exact-match id constraint is encoded as extra one-hot "digit" dimensions
    (base-32 digits of the id, scaled by 64) plus a -8192 penalty constant:
    equal ids add 0 to the score, any mismatch adds <= -4096, so
    relu(max_j score) == max_j (score * exact_match) for this data regime.
  - doc projection and token-score matmuls interleave per doc-chunk to keep
    the PE dense (HAM at full clock; dummy warmup matmuls pre-promote the
    clock while DMAs stream); DVE does paired [128,8,128] max reduces
    straight from PSUM (reduce rate is 1x regardless of staging, so the
    ~39us DVE scan is the steady-state bound and everything overlaps it).
  - each doc-pair's finished maxes fold into the final scores immediately
    (selector matmul accumulating onto the CLS matmuls in one PSUM tile).
Host: pre-packs bf16 operands in SBUF layout (one <=128-descriptor DMA per
tensor), gathers per-core [32,32] score tiles, computes softmax loss.
"""

import os
import numpy as np
import ml_dtypes

import concourse.bass as bass
import concourse.bacc as bacc
import concourse.mybir as mybir
from concourse import tile
from concourse.bass_utils import run_bass_kernel_spmd

BF16 = mybir.dt.bfloat16
FP8 = mybir.dt.float8e4
F32 = mybir.dt.float32
RELU = mybir.ActivationFunctionType.Relu
IDENT = mybir.ActivationFunctionType.Identity
COPY = mybir.ActivationFunctionType.Copy
AX_X = mybir.AxisListType.X
PSUM = bass.MemorySpace.PSUM

N_CORES = 8
Bq, Lq, Bd, Ld, H, Dt, Dc = 32, 32, 256, 128, 768, 32, 768
DPC = Bd // N_CORES          # docs per core = 32
NDT = DPC * Ld               # doc tokens per core = 4096
NQT = Bq * Lq                # query tokens = 1024
KC = H // 128                # contraction chunks = 6
EXT = 65                     # id-encoding extension rows (32+32 digits + const)
KTOT = Dt + EXT              # 97
CONE = 64.0                  # one-hot scale; CONE^2 = 4096 penalty unit
PEN = -2.0 * CONE * CONE     # -8192
NT = NQT // 128              # query-token tiles = 8
NN = NDT // 512              # doc-token 512-chunks = 8
NP = NN // 2                 # paired 1024-chunks = 4
TRAIN_GROUP_SIZE = 8

bf16 = ml_dtypes.bfloat16
fp8 = ml_dtypes.float8_e4m3

_prog = None
last_results = None          # BassKernelResults of the most recent run


def _build():
    nc = bacc.Bacc("TRN2", target_bir_lowering=False, debug=False)

    dhT_d = nc.dram_tensor("dhT", [128, NN, KC, 512], BF16, kind="ExternalInput")
    qhT_d = nc.dram_tensor("qhT", [128, KC, NQT], BF16, kind="ExternalInput")
    h0T_d = nc.dram_tensor("h0T", [128, KC, 2 * Bq], BF16, kind="ExternalInput")
    wtok_d = nc.dram_tensor("wtok", [128, KC, Dt], BF16, kind="ExternalInput")
    wcls_d = nc.dram_tensor("wcls", [128, KC, Dc], BF16, kind="ExternalInput")
    btok_d = nc.dram_tensor("btok", [Dt, 1], F32, kind="ExternalInput")
    bcls_d = nc.dram_tensor("bcls", [128, KC], F32, kind="ExternalInput")
    extq_d = nc.dram_tensor("extq", [EXT, NQT], BF16, kind="ExternalInput")
    extd_d = nc.dram_tensor("extd", [EXT, NDT], BF16, kind="ExternalInput")
    wsel_d = nc.dram_tensor("wsel", [128, NT, Bq], BF16, kind="ExternalInput")
    ident_d = nc.inline_tensor(np.eye(2 * Bq, dtype=bf16), name="ident64")
    out_d = nc.dram_tensor("scores_out", [Bq, DPC], F32, kind="ExternalOutput")

    with tile.TileContext(nc) as tc:
        with (
            tc.tile_pool(name="big", bufs=1) as bigp,
            tc.tile_pool(name="psP", bufs=2, space=PSUM) as psPp,
            tc.tile_pool(name="psS", bufs=2, space=PSUM) as psSp,
            tc.tile_pool(name="psC", bufs=1, space=PSUM) as psCp,
            tc.tile_pool(name="psF", bufs=1, space=PSUM) as psFp,
        ):
            # ---- loads: host-packed, one DMA per tensor, priority order.
            # sync ring: small consts -> wcls (cls unblocks PE early) -> qhT
            wtok_sb = bigp.tile([128, KC, Dt], BF16, tag="wtok")
            wcls_sb = bigp.tile([128, KC, Dc], BF16, tag="wcls")
            h0T_sb = bigp.tile([128, KC, 2 * Bq], BF16, tag="h0T")
            btok_sb = bigp.tile([Dt, 1], F32, tag="btok")
            bcls_sb = bigp.tile([128, KC], F32, tag="bcls")
            wsel_sb = bigp.tile([128, NT, Bq], BF16, tag="wsel")
            ident_sb = bigp.tile([2 * Bq, 2 * Bq], BF16, tag="ident")
            QT = bigp.tile([128, NQT], BF16, tag="QT")
            qhT_sb = bigp.tile([128, KC, NQT], BF16, tag="qhT")
            DT = bigp.tile([128, NDT], BF16, tag="DT")
            dhq = bigp.tile([128, NN, KC, 512], BF16, tag="dh")
            nc.sync.dma_start(out=wtok_sb[:, :, :], in_=wtok_d[:, :, :])
            nc.sync.dma_start(out=h0T_sb[:, :, :], in_=h0T_d[:, :, :])
            nc.sync.dma_start(out=btok_sb[:, :], in_=btok_d[:, :])
            nc.sync.dma_start(out=bcls_sb[:, :], in_=bcls_d[:, :])
            nc.sync.dma_start(out=ident_sb[:, :], in_=ident_d[:, :])
            nc.sync.dma_start(out=qhT_sb[:, :, :], in_=qhT_d[:, :, :])
            nc.sync.dma_start(out=QT[Dt : Dt + EXT, :], in_=extq_d[:, :])
            nc.sync.dma_start(out=dhq[:, 0, :, :], in_=dhT_d[:, 0, :, :])
            nc.sync.dma_start(out=DT[Dt : Dt + EXT, :], in_=extd_d[:, :])
            nc.sync.dma_start(out=dhq[:, 1, :, :], in_=dhT_d[:, 1, :, :])
            nc.sync.dma_start(out=dhq[:, 2:4, :, :], in_=dhT_d[:, 2:4, :, :])
            nc.sync.dma_start(out=dhq[:, 4:6, :, :], in_=dhT_d[:, 4:6, :, :])
            nc.sync.dma_start(out=wcls_sb[:, :, :], in_=wcls_d[:, :, :])
            nc.sync.dma_start(out=dhq[:, 6:8, :, :], in_=dhT_d[:, 6:8, :, :])
            nc.sync.dma_start(out=wsel_sb[:, :, :], in_=wsel_d[:, :, :])

            # ---- PE warmup: promote HAM to full clock while DMAs stream
            for wi in range(24):
                psw = psPp.tile([Dt, 512], F32, tag="proj")
                nc.tensor.matmul(
                    psw[:, 0:192],
                    lhsT=wtok_sb[:, 0, :],
                    rhs=wtok_sb[:, :, :].rearrange("x c n -> x (c n)"),
                    start=True,
                    stop=True,
                )

            # ---- query projection (first half; second half lands mid-stream)
            def qproj(h):
                ps = psPp.tile([Dt, 512], F32, tag="proj")
                hs = slice(h * 512, (h + 1) * 512)
                for c in range(KC):
                    nc.tensor.matmul(
                        ps[:, :],
                        lhsT=wtok_sb[:, c, :],
                        rhs=qhT_sb[:, c, hs],
                        start=(c == 0),
                        stop=(c == KC - 1),
                    )
                nc.scalar.activation(QT[0:Dt, hs], ps[:, :], RELU, bias=btok_sb[:, 0:1])

            qproj(0)

            # ---- interleaved doc projection + token scores.
            # Groups: two 512-col singles (so DVE starts as soon as the first
            # eighth of doc hidden lands), then three 1024-col pairs.
            tokraw = bigp.tile([128, NT, DPC], BF16, tag="tokraw")
            tok = bigp.tile([128, NT, DPC], BF16, tag="tok")
            psF = psFp.tile([Bq, DPC], F32, tag="fin")
            qd2 = bigp.tile([2 * Bq, Dc], BF16, tag="qd2")
            qdcls = bigp.tile([128, KC, 2 * Bq], BF16, tag="qdcls")
            groups = [(0,), (1,), (2, 3), (4, 5), (6, 7)]

            def dproj(grp):
                for n in grp:
                    ps = psPp.tile([Dt, 512], F32, tag="proj")
                    for c in range(KC):
                        nc.tensor.matmul(
                            ps[:, :],
                            lhsT=wtok_sb[:, c, :],
                            rhs=dhq[:, n, c, :],
                            start=(c == 0),
                            stop=(c == KC - 1),
                        )
                    nc.scalar.activation(
                        DT[0:Dt, n * 512 : (n + 1) * 512],
                        ps[:, :],
                        RELU,
                        bias=btok_sb[:, 0:1],
                    )

            dproj(groups[0])
            for gi, grp in enumerate(groups):
                for t in range(NT):
                    if gi == 0 and t == 4:
                        qproj(1)
                    ts = slice(t * 128, (t + 1) * 128)
                    psb = psSp.tile([128, 512 * len(grp)], F32, tag="S")
                    for j, n in enumerate(grp):
                        nc.tensor.matmul(
                            psb[:, j * 512 : (j + 1) * 512],
                            lhsT=QT[0:KTOT, ts],
                            rhs=DT[0:KTOT, n * 512 : (n + 1) * 512],
                            start=True,
                            stop=True,
                        )
                    nc.vector.reduce_max(
                        tokraw[:, t, grp[0] * 4 : (grp[-1] + 1) * 4],
                        psb[:, :].rearrange("x (a b) -> x a b", b=Ld),
                        axis=AX_X,
                    )
                if gi + 1 < len(groups):
                    dproj(groups[gi + 1])
                if gi == 0:
                    continue
                # fold the completed 8-doc block into the final scores
                b = gi - 1
                for t in range(NT):
                    psl = slice(b * 8, (b + 1) * 8)
                    nc.scalar.activation(tok[:, t, psl], tokraw[:, t, psl], RELU)
                    nc.tensor.matmul(
                        psF[:, psl],
                        lhsT=wsel_sb[:, t, :],
                        rhs=tok[:, t, psl],
                        start=(b == 0 and t == 0),
                        stop=(b == 3 and t == NT - 1),
                    )
                    if b == 3 and t == 0:
                        # cls scores accumulate here, off the critical tail
                        for m in range(KC):
                            nc.tensor.matmul(
                                psF[:, :],
                                lhsT=qdcls[:, m, 0:Bq],
                                rhs=qdcls[:, m, Bq : 2 * Bq],
                                start=False,
                                stop=False,
                            )
                if gi == 2:
                    # cls projection filler (wcls arrives during the S stream)
                    for half in range(2):
                        psc = psCp.tile([2 * Bq, Dc // 2], F32, tag="mix")
                        cs = slice(half * 384, (half + 1) * 384)
                        for c in range(KC):
                            nc.tensor.matmul(
                                psc[:, :],
                                lhsT=h0T_sb[:, c, :],
                                rhs=wcls_sb[:, c, cs],
                                start=(c == 0),
                                stop=(c == KC - 1),
                            )
                        nc.scalar.activation(qd2[:, cs], psc[:, :], COPY)
                elif gi == 3:
                    # cls transpose filler
                    for m in range(KC):
                        pst = psPp.tile([128, 2 * Bq], BF16, tag="proj")
                        nc.tensor.transpose(
                            pst[:, :], qd2[:, m * 128 : (m + 1) * 128], ident_sb[:, :]
                        )
                        nc.scalar.activation(
                            qdcls[:, m, :], pst[:, :], IDENT, bias=bcls_sb[:, m : m + 1]
                        )

            scr = bigp.tile([Bq, DPC], F32, tag="scr")
            nc.vector.tensor_copy(scr[:, :], psF[:, :])
            nc.sync.dma_start(out=out_d[:, :], in_=scr[:, :])

    nc.compile()
    return nc


def _get_prog():
    global _prog
    if _prog is None:
        _prog = _build()
    return _prog


def _pack(a, p=128):
    """[C*p, N...] -> [p, C, N...] contiguous (SBUF partition-major layout)."""
    c = a.shape[0] // p
    return np.ascontiguousarray(a.reshape((c, p) + a.shape[1:]).swapaxes(0, 1))


def _prep_inputs(qry_hidden, doc_hidden, W_tok, b_tok, W_cls, b_cls,
                 qry_input_ids, doc_input_ids, qry_attention_mask):
    qh = np.asarray(qry_hidden, np.float32)
    dh = np.asarray(doc_hidden, np.float32)
    qids = np.asarray(qry_input_ids, np.int32).reshape(-1)
    dids = np.asarray(doc_input_ids, np.int32)
    amask = np.asarray(qry_attention_mask, np.int32)

    qhT = np.ascontiguousarray(qh.reshape(NQT, H).astype(bf16).T)     # [768, 1024]
    wtok = _pack(np.asarray(W_tok, np.float32).astype(bf16))          # [128,6,32]
    wcls = _pack(np.asarray(W_cls, np.float32).astype(bf16))          # [128,6,768]
    btok = np.asarray(b_tok, np.float32).reshape(Dt, 1)
    bcls = _pack(np.asarray(b_cls, np.float32).reshape(Dc, 1))[:, :, 0]  # [128,6]

    g = np.arange(NQT)
    extq = np.zeros((EXT, NQT), np.float32)
    extq[qids % 32, g] = CONE
    extq[32 + qids // 32, g] = CONE
    extq[64, :] = 1.0
    extq = extq.astype(bf16)

    # query-token weights: qmask with sep position zeroed, i=0 dropped
    sep = amask.sum(1) - 1
    qm = amask.astype(np.float32).copy()
    qm[np.arange(Bq), sep] = 0.0
    w = qm.copy()
    w[:, 0] = 0.0
    wsel = np.zeros((NQT, Bq), np.float32)
    wsel[g, g // Lq] = w.reshape(-1)
    wsel = _pack(wsel.astype(bf16))                                   # [128,8,32]

    qh0 = qh[:, 0, :]
    in_maps = []
    for k in range(N_CORES):
        dsl = slice(k * DPC, (k + 1) * DPC)
        dh_k = dh[dsl].reshape(NDT, H)
        dhT_k = dh_k.astype(bf16).T                                   # [768, 4096]
        # pack to [128, 8(eighth), 6(c), 512]
        dhTP = np.ascontiguousarray(
            dhT_k.reshape(KC, 128, NN, 512).transpose(1, 2, 0, 3)
        )
        h0T_k = _pack(
            np.ascontiguousarray(
                np.concatenate([qh0, dh[dsl, 0, :]], axis=0).astype(bf16).T
            )
        )                                                             # [128,6,64]
        dids_k = dids[dsl].reshape(-1)
        gd = np.arange(NDT)
        extd = np.zeros((EXT, NDT), np.float32)
        extd[dids_k % 32, gd] = CONE
        extd[32 + dids_k // 32, gd] = CONE
        extd[64, :] = PEN
        in_maps.append({
            "dhT": dhTP,
            "qhT": _pack(qhT),
            "h0T": h0T_k,
            "wtok": wtok,
            "wcls": wcls,
            "btok": btok,
            "bcls": bcls,
            "extq": extq,
            "extd": extd.astype(bf16),
            "wsel": wsel,
        })
    return in_maps


def kernel(**inputs):
    global last_results
    nc = _get_prog()
    in_maps = _prep_inputs(**inputs)
    trace = bool(os.environ.get("COIL_TRACE"))
    last_results = run_bass_kernel_spmd(
        nc, in_maps, list(range(N_CORES)), trace=trace
    )
    scores = np.concatenate(
        [last_results.results[k]["scores_out"] for k in range(N_CORES)], axis=1
    ).astype(np.float64)

    labels = np.arange(Bq) * TRAIN_GROUP_SIZE
    m = scores.max(axis=1, keepdims=True)
    lse = m[:, 0] + np.log(np.exp(scores - m).sum(axis=1))
    loss = -(scores[np.arange(Bq), labels] - lse).mean()
    return (
        np.asarray(loss, np.float32),
        scores.reshape(-1).astype(np.float32),
    )


# revision 24
# speedup vs baseline: 1.0689x; 1.0395x over previous
"""COIL scoring kernel for Trainium2, sharded over 8 NeuronCores.

Sharding: data-parallel over documents (Bd=256 -> 32 docs/core). Each core:
  - projects its doc tokens with W_tok (+ReLU) and token-0 with W_cls
  - projects the (replicated) query tokens the same way
  - computes token-level scores via [97 x 128 x 512] matmuls where the
    exact-match id constraint is encoded as extra one-hot "digit" dimensions
    (base-32 digits of the id, scaled by 64) plus a -8192 penalty constant:
    equal ids add 0 to the score, any mismatch adds <= -4096, so
    relu(max_j score) == max_j (score * exact_match) for this data regime.
  - doc projection and token-score matmuls interleave per doc-chunk to keep
    the PE dense (HAM at full clock; dummy warmup matmuls pre-promote the
    clock while DMAs stream); DVE does paired [128,8,128] max reduces
    straight from PSUM (reduce rate is 1x regardless of staging, so the
    ~39us DVE scan is the steady-state bound and everything overlaps it).
  - each doc-pair's finished maxes fold into the final scores immediately
    (selector matmul accumulating onto the CLS matmuls in one PSUM tile).
Host: pre-packs bf16 operands in SBUF layout (one <=128-descriptor DMA per
tensor), gathers per-core [32,32] score tiles, computes softmax loss.
"""

import os
import numpy as np
import ml_dtypes

import concourse.bass as bass
import concourse.bacc as bacc
import concourse.mybir as mybir
from concourse import tile
from concourse.bass_utils import run_bass_kernel_spmd

BF16 = mybir.dt.bfloat16
FP8 = mybir.dt.float8e4
F32 = mybir.dt.float32
RELU = mybir.ActivationFunctionType.Relu
IDENT = mybir.ActivationFunctionType.Identity
COPY = mybir.ActivationFunctionType.Copy
AX_X = mybir.AxisListType.X
PSUM = bass.MemorySpace.PSUM

N_CORES = 8
Bq, Lq, Bd, Ld, H, Dt, Dc = 32, 32, 256, 128, 768, 32, 768
DPC = Bd // N_CORES          # docs per core = 32
NDT = DPC * Ld               # doc tokens per core = 4096
NQT = Bq * Lq                # query tokens = 1024
KC = H // 128                # contraction chunks = 6
EXT = 65                     # id-encoding extension rows (32+32 digits + const)
KTOT = Dt + EXT              # 97
CONE = 64.0                  # one-hot scale; CONE^2 = 4096 penalty unit
PEN = -2.0 * CONE * CONE     # -8192
NT = NQT // 128              # query-token tiles = 8
NN = NDT // 512              # doc-token 512-chunks = 8
NP = NN // 2                 # paired 1024-chunks = 4
TRAIN_GROUP_SIZE = 8

bf16 = ml_dtypes.bfloat16
fp8 = ml_dtypes.float8_e4m3

_prog = None
last_results = None          # BassKernelResults of the most recent run


def _build():
    nc = bacc.Bacc("TRN2", target_bir_lowering=False, debug=False)

    dhT_d = nc.dram_tensor("dhT", [128, NN, KC, 512], BF16, kind="ExternalInput")
    qhT_d = nc.dram_tensor("qhT", [128, KC, NQT], BF16, kind="ExternalInput")
    h0T_d = nc.dram_tensor("h0T", [128, KC, 2 * Bq], BF16, kind="ExternalInput")
    wtok_d = nc.dram_tensor("wtok", [128, KC, Dt], BF16, kind="ExternalInput")
    wcls_d = nc.dram_tensor("wcls", [128, KC, Dc], BF16, kind="ExternalInput")
    btok_d = nc.dram_tensor("btok", [Dt, 1], F32, kind="ExternalInput")
    bcls_d = nc.dram_tensor("bcls", [128, KC], F32, kind="ExternalInput")
    extq_d = nc.dram_tensor("extq", [EXT, NQT], BF16, kind="ExternalInput")
    extd_d = nc.dram_tensor("extd", [EXT, NDT], BF16, kind="ExternalInput")
    wsel_d = nc.dram_tensor("wsel", [128, NT, Bq], BF16, kind="ExternalInput")
    ident_d = nc.inline_tensor(np.eye(2 * Bq, dtype=bf16), name="ident64")
    out_d = nc.dram_tensor("scores_out", [Bq, DPC], F32, kind="ExternalOutput")

    with tile.TileContext(nc) as tc:
        with (
            tc.tile_pool(name="big", bufs=1) as bigp,
            tc.tile_pool(name="psP", bufs=2, space=PSUM) as psPp,
            tc.tile_pool(name="psS", bufs=2, space=PSUM) as psSp,
            tc.tile_pool(name="psC", bufs=1, space=PSUM) as psCp,
            tc.tile_pool(name="psF", bufs=1, space=PSUM) as psFp,
        ):
            # ---- loads: host-packed, one DMA per tensor, priority order.
            # sync ring: small consts -> wcls (cls unblocks PE early) -> qhT
            wtok_sb = bigp.tile([128, KC, Dt], BF16, tag="wtok")
            wcls_sb = bigp.tile([128, KC, Dc], BF16, tag="wcls")
            h0T_sb = bigp.tile([128, KC, 2 * Bq], BF16, tag="h0T")
            btok_sb = bigp.tile([Dt, 1], F32, tag="btok")
            bcls_sb = bigp.tile([128, KC], F32, tag="bcls")
            wsel_sb = bigp.tile([128, NT, Bq], BF16, tag="wsel")
            ident_sb = bigp.tile([2 * Bq, 2 * Bq], BF16, tag="ident")
            QT = bigp.tile([128, NQT], BF16, tag="QT")
            qhT_sb = bigp.tile([128, KC, NQT], BF16, tag="qhT")
            DT = bigp.tile([128, NDT], BF16, tag="DT")
            dhq = bigp.tile([128, NN, KC, 512], BF16, tag="dh")
            nc.sync.dma_start(out=wtok_sb[:, :, :], in_=wtok_d[:, :, :])
            nc.sync.dma_start(out=btok_sb[:, :], in_=btok_d[:, :])
            nc.sync.dma_start(out=qhT_sb[:, :, :], in_=qhT_d[:, :, :])
            nc.sync.dma_start(out=QT[Dt : Dt + EXT, :], in_=extq_d[:, :])
            nc.sync.dma_start(out=dhq[:, 0, :, :], in_=dhT_d[:, 0, :, :])
            nc.sync.dma_start(out=DT[Dt : Dt + EXT, :], in_=extd_d[:, :])
            nc.sync.dma_start(out=dhq[:, 1, :, :], in_=dhT_d[:, 1, :, :])
            nc.sync.dma_start(out=dhq[:, 2:4, :, :], in_=dhT_d[:, 2:4, :, :])
            nc.sync.dma_start(out=dhq[:, 4:6, :, :], in_=dhT_d[:, 4:6, :, :])
            nc.sync.dma_start(out=h0T_sb[:, :, :], in_=h0T_d[:, :, :])
            nc.sync.dma_start(out=bcls_sb[:, :], in_=bcls_d[:, :])
            nc.sync.dma_start(out=ident_sb[:, :], in_=ident_d[:, :])
            nc.sync.dma_start(out=wcls_sb[:, :, :], in_=wcls_d[:, :, :])
            nc.sync.dma_start(out=dhq[:, 6:8, :, :], in_=dhT_d[:, 6:8, :, :])
            nc.sync.dma_start(out=wsel_sb[:, :, :], in_=wsel_d[:, :, :])

            # ---- PE warmup: promote HAM to full clock while DMAs stream
            for wi in range(24):
                psw = psPp.tile([Dt, 512], F32, tag="proj")
                nc.tensor.matmul(
                    psw[:, 0:192],
                    lhsT=wtok_sb[:, 0, :],
                    rhs=wtok_sb[:, :, :].rearrange("x c n -> x (c n)"),
                    start=True,
                    stop=True,
                )

            # ---- query projection (first half; second half lands mid-stream)
            def qproj(h):
                ps = psPp.tile([Dt, 512], F32, tag="proj")
                hs = slice(h * 512, (h + 1) * 512)
                for c in range(KC):
                    nc.tensor.matmul(
                        ps[:, :],
                        lhsT=wtok_sb[:, c, :],
                        rhs=qhT_sb[:, c, hs],
                        start=(c == 0),
                        stop=(c == KC - 1),
                    )
                nc.scalar.activation(QT[0:Dt, hs], ps[:, :], RELU, bias=btok_sb[:, 0:1])

            qproj(0)

            # ---- interleaved doc projection + token scores.
            # Groups: two 512-col singles (so DVE starts as soon as the first
            # eighth of doc hidden lands), then three 1024-col pairs.
            tokraw = bigp.tile([128, NT, DPC], BF16, tag="tokraw")
            tok = bigp.tile([128, NT, DPC], BF16, tag="tok")
            psF = psFp.tile([Bq, DPC], F32, tag="fin")
            qd2 = bigp.tile([2 * Bq, Dc], BF16, tag="qd2")
            qdcls = bigp.tile([128, KC, 2 * Bq], BF16, tag="qdcls")
            groups = [(0,), (1,), (2, 3), (4, 5), (6, 7)]

            def dproj(grp):
                for n in grp:
                    ps = psPp.tile([Dt, 512], F32, tag="proj")
                    for c in range(KC):
                        nc.tensor.matmul(
                            ps[:, :],
                            lhsT=wtok_sb[:, c, :],
                            rhs=dhq[:, n, c, :],
                            start=(c == 0),
                            stop=(c == KC - 1),
                        )
                    nc.scalar.activation(
                        DT[0:Dt, n * 512 : (n + 1) * 512],
                        ps[:, :],
                        RELU,
                        bias=btok_sb[:, 0:1],
                    )

            dproj(groups[0])
            for gi, grp in enumerate(groups):
                for t in range(NT):
                    if gi == 0 and t == 4:
                        qproj(1)
                    ts = slice(t * 128, (t + 1) * 128)
                    psb = psSp.tile([128, 512 * len(grp)], F32, tag="S")
                    for j, n in enumerate(grp):
                        nc.tensor.matmul(
                            psb[:, j * 512 : (j + 1) * 512],
                            lhsT=QT[0:KTOT, ts],
                            rhs=DT[0:KTOT, n * 512 : (n + 1) * 512],
                            start=True,
                            stop=True,
                        )
                    nc.vector.reduce_max(
                        tokraw[:, t, grp[0] * 4 : (grp[-1] + 1) * 4],
                        psb[:, :].rearrange("x (a b) -> x a b", b=Ld),
                        axis=AX_X,
                    )
                if gi + 1 < len(groups):
                    dproj(groups[gi + 1])
                if gi == 0:
                    continue
                # fold the completed 8-doc block into the final scores
                b = gi - 1
                for t in range(NT):
                    psl = slice(b * 8, (b + 1) * 8)
                    nc.scalar.activation(tok[:, t, psl], tokraw[:, t, psl], RELU)
                    nc.tensor.matmul(
                        psF[:, psl],
                        lhsT=wsel_sb[:, t, :],
                        rhs=tok[:, t, psl],
                        start=(b == 0 and t == 0),
                        stop=(b == 3 and t == NT - 1),
                    )
                    if b == 3 and t == 0:
                        # cls scores accumulate here, off the critical tail
                        for m in range(KC):
                            nc.tensor.matmul(
                                psF[:, :],
                                lhsT=qdcls[:, m, 0:Bq],
                                rhs=qdcls[:, m, Bq : 2 * Bq],
                                start=False,
                                stop=False,
                            )
                if gi == 2:
                    # cls projection filler (wcls arrives during the S stream)
                    for half in range(2):
                        psc = psCp.tile([2 * Bq, Dc // 2], F32, tag="mix")
                        cs = slice(half * 384, (half + 1) * 384)
                        for c in range(KC):
                            nc.tensor.matmul(
                                psc[:, :],
                                lhsT=h0T_sb[:, c, :],
                                rhs=wcls_sb[:, c, cs],
                                start=(c == 0),
                                stop=(c == KC - 1),
                            )
                        nc.scalar.activation(qd2[:, cs], psc[:, :], COPY)
                elif gi == 3:
                    # cls transpose filler
                    for m in range(KC):
                        pst = psPp.tile([128, 2 * Bq], BF16, tag="proj")
                        nc.tensor.transpose(
                            pst[:, :], qd2[:, m * 128 : (m + 1) * 128], ident_sb[:, :]
                        )
                        nc.scalar.activation(
                            qdcls[:, m, :], pst[:, :], IDENT, bias=bcls_sb[:, m : m + 1]
                        )

            scr = bigp.tile([Bq, DPC], F32, tag="scr")
            nc.vector.tensor_copy(scr[:, :], psF[:, :])
            nc.sync.dma_start(out=out_d[:, :], in_=scr[:, :])

    nc.compile()
    return nc


def _get_prog():
    global _prog
    if _prog is None:
        _prog = _build()
    return _prog


def _pack(a, p=128):
    """[C*p, N...] -> [p, C, N...] contiguous (SBUF partition-major layout)."""
    c = a.shape[0] // p
    return np.ascontiguousarray(a.reshape((c, p) + a.shape[1:]).swapaxes(0, 1))


def _prep_inputs(qry_hidden, doc_hidden, W_tok, b_tok, W_cls, b_cls,
                 qry_input_ids, doc_input_ids, qry_attention_mask):
    qh = np.asarray(qry_hidden, np.float32)
    dh = np.asarray(doc_hidden, np.float32)
    qids = np.asarray(qry_input_ids, np.int32).reshape(-1)
    dids = np.asarray(doc_input_ids, np.int32)
    amask = np.asarray(qry_attention_mask, np.int32)

    qhT = np.ascontiguousarray(qh.reshape(NQT, H).astype(bf16).T)     # [768, 1024]
    wtok = _pack(np.asarray(W_tok, np.float32).astype(bf16))          # [128,6,32]
    wcls = _pack(np.asarray(W_cls, np.float32).astype(bf16))          # [128,6,768]
    btok = np.asarray(b_tok, np.float32).reshape(Dt, 1)
    bcls = _pack(np.asarray(b_cls, np.float32).reshape(Dc, 1))[:, :, 0]  # [128,6]

    g = np.arange(NQT)
    extq = np.zeros((EXT, NQT), np.float32)
    extq[qids % 32, g] = CONE
    extq[32 + qids // 32, g] = CONE
    extq[64, :] = 1.0
    extq = extq.astype(bf16)

    # query-token weights: qmask with sep position zeroed, i=0 dropped
    sep = amask.sum(1) - 1
    qm = amask.astype(np.float32).copy()
    qm[np.arange(Bq), sep] = 0.0
    w = qm.copy()
    w[:, 0] = 0.0
    wsel = np.zeros((NQT, Bq), np.float32)
    wsel[g, g // Lq] = w.reshape(-1)
    wsel = _pack(wsel.astype(bf16))                                   # [128,8,32]

    qh0 = qh[:, 0, :]
    in_maps = []
    for k in range(N_CORES):
        dsl = slice(k * DPC, (k + 1) * DPC)
        dh_k = dh[dsl].reshape(NDT, H)
        dhT_k = dh_k.astype(bf16).T                                   # [768, 4096]
        # pack to [128, 8(eighth), 6(c), 512]
        dhTP = np.ascontiguousarray(
            dhT_k.reshape(KC, 128, NN, 512).transpose(1, 2, 0, 3)
        )
        h0T_k = _pack(
            np.ascontiguousarray(
                np.concatenate([qh0, dh[dsl, 0, :]], axis=0).astype(bf16).T
            )
        )                                                             # [128,6,64]
        dids_k = dids[dsl].reshape(-1)
        gd = np.arange(NDT)
        extd = np.zeros((EXT, NDT), np.float32)
        extd[dids_k % 32, gd] = CONE
        extd[32 + dids_k // 32, gd] = CONE
        extd[64, :] = PEN
        in_maps.append({
            "dhT": dhTP,
            "qhT": _pack(qhT),
            "h0T": h0T_k,
            "wtok": wtok,
            "wcls": wcls,
            "btok": btok,
            "bcls": bcls,
            "extq": extq,
            "extd": extd.astype(bf16),
            "wsel": wsel,
        })
    return in_maps


def kernel(**inputs):
    global last_results
    nc = _get_prog()
    in_maps = _prep_inputs(**inputs)
    trace = bool(os.environ.get("COIL_TRACE"))
    last_results = run_bass_kernel_spmd(
        nc, in_maps, list(range(N_CORES)), trace=trace
    )
    scores = np.concatenate(
        [last_results.results[k]["scores_out"] for k in range(N_CORES)], axis=1
    ).astype(np.float64)

    labels = np.arange(Bq) * TRAIN_GROUP_SIZE
    m = scores.max(axis=1, keepdims=True)
    lse = m[:, 0] + np.log(np.exp(scores - m).sum(axis=1))
    loss = -(scores[np.arange(Bq), labels] - lse).mean()
    return (
        np.asarray(loss, np.float32),
        scores.reshape(-1).astype(np.float32),
    )
